# revision 4
# baseline (speedup 1.0000x reference)
"""Trainium2 Bass kernel for nn_AttentionSlice (non-local attention block).

Reference computation (B=4, C=128, Ci=64, H=W=64, N=H*W=4096):
  theta = BN(conv1x1(x1))   [B, Ci, N]
  phi   = BN(conv1x1(x2))   [B, Ci, N]
  g     = BN(conv1x1(x2))   [B, Ci, N]
  attn  = softmax(theta^T @ phi, axis=-1)          [B, N, N]
  out   = BN(conv1x1(attn @ g^T))                  [B, C, H, W]
  return concat([out, x1], axis=1)                 [B, 2C, H, W]

Sharding: 8 cores = 4 batch samples x 2 halves of the N attention rows.
Each core computes a [2048, 4096] attention block; no cross-core comms.

Device-side design (per core, n = this core's 2048 attention rows):
  - BatchNorm folded into conv weights on the host.
  - S^T is computed chunk-wise in [m, n] layout (m = key position on
    partitions) so softmax needs NO transposes: exp is applied directly
    (S max ~66 < 88, so no max-subtraction is needed for fp32 exp), and
    the softmax denominator falls out of the attn@g^T matmul via an
    appended ones-column on g^T. Division is deferred through the final
    1x1 conv (scaling commutes with the channel contraction).
  - Mixed precision on the hot path (measured fastest on HW): fp16 for
    theta/phi (10-bit mantissa — S errors are exponentiated by softmax so
    precision matters; values are small so fp16 range suffices), bf16 for
    exp(S) and g^T (need fp32-scale exponent range; they enter a weighted
    average so rounding mostly cancels), float32r projections, fp32 tail.
    End-to-end L2 relative error vs the fp32 reference: ~2e-3.
  - Even/odd m-chunks use PE row-groups 0/1 (tile_position) so their
    S^T matmuls run concurrently on the half-idle (K=64) PE array.
  - Output is returned in z^T layout [n, c]; the host transposes and
    adds the (BN-folded) output bias while unsharding.
"""

import sys

if "/opt/trn_rl_repo" not in sys.path:
    sys.path.insert(0, "/opt/trn_rl_repo")

import numpy as np

import concourse.bacc as bacc
import concourse.mybir as mybir
import concourse.tile as tile
from concourse.bass_utils import run_bass_kernel_spmd
import os as _os


def _enable_ldw_opt():
    """Re-enable walrus LDWEIGHTS elision (skips redundant weight loads when
    consecutive matmuls share lhsT). bass_utils hardcodes it off; it breaks
    For_i-loop codegen so it is applied only for the straight-line kernel."""
    import concourse.bass_utils as _bu

    if getattr(_bu, "_ldw_opt_patched", False):
        return
    _orig_run_command = _bu.run_command

    def _run_command_ldwopt(argv, **kw):
        argv = [
            "--enable-ldw-opt=true" if a == "--enable-ldw-opt=false" else a
            for a in argv
        ]
        return _orig_run_command(argv, **kw)

    _bu.run_command = _run_command_ldwopt
    _bu._ldw_opt_patched = True


if _os.environ.get("KLDW", "0") == "1":
    _enable_ldw_opt()

EPS = 1e-5
B, C, CI, H, W = 4, 128, 64, 64, 64
N = H * W  # 4096
NCORES = 8
NH = N // 2  # 2048 rows of attention per core
HALF = 1024  # n processed per pass (PSUM budget)
NCHUNK = 32  # m chunks of 128

F32 = mybir.dt.float32
F32R = mybir.dt.float32r
Exp = mybir.ActivationFunctionType.Exp

# blob column layout: constants first (small fast DMA), then xa.
WTH = 0  # [128, 128] doubled theta weights (lhsT)
WPH = WTH + 128  # [128, 128] doubled phi weights
WG = WPH + 128  # [128, 64]  g weights (rhs form)
WO = WG + 64  # [128, 128] wout weights, rows 0:64 valid (rhs form)
BTH = WO + 128  # [128, 1] theta bias (doubled)
BPH = BTH + 1  # [128, 1] phi bias (doubled)
ONE = BPH + 1  # [128, 1] all ones
ONES32 = ONE + 1  # [128, 32] ones (DMA'd into gta's ones-columns)
CONST_W = ONES32 + 32  # 483
XA = CONST_W  # [128, 2048] x1 slice
BLOB_W = XA + NH

_CACHE: dict = {}


def _build_skeleton(reps: int):
    """Same rough instruction mix as the real kernel, tiny data: measures
    per-instruction issue/sync overhead on HW."""
    nc = bacc.Bacc(trn_type="TRN2")
    blob_d = nc.dram_tensor("blob", [128, BLOB_W], F32R, kind="ExternalInput")
    xb_d = nc.dram_tensor("xb", [128, N], F32R, kind="ExternalInput")
    out_d = nc.dram_tensor("out", [NH, 128], F32, kind="ExternalOutput")
    with tile.TileContext(nc) as tc:
        with tc.tile_pool(name="sb", bufs=1) as sb, tc.tile_pool(
            name="ps", bufs=4, space="PSUM"
        ) as ps:
            a = sb.tile([128, 64], F32R, name="a")
            nc.sync.dma_start(a[:, 0:32], blob_d[:, 0:32])
            nc.sync.dma_start(a[:, 32:64], blob_d[:, 32:64])
            o = sb.tile([128, 64], F32, name="o")
            with tc.For_i(0, reps, 1, hint_engines=(mybir.EngineType.PE, mybir.EngineType.Activation, mybir.EngineType.DVE)):
                for i in range(8):
                    nc.sync.dma_start(a[:, 8 * i : 8 * i + 8], blob_d[:, 8 * i : 8 * i + 8])
                for jj in range(68):
                    p = ps.tile([128, 16], F32, name=f"p{jj}", tag="p")
                    for k in range(5):
                        nc.tensor.matmul(
                            p[0 : 2 + k, 2 * k : 2 * k + 2],
                            a[0:2, k : k + 2 + k].bitcast(F32R),
                            a[0:2, 8 : 8 + 2].bitcast(F32R),
                            start=True,
                            stop=True,
                        )
                    e = sb.tile([128, 16], F32, name=f"e{jj}", tag="e", bufs=4)
                    nc.scalar.activation(e[0:2, 0:2], p[0:2, 0:2], Exp)
                    if jj % 2 == 0:
                        nc.vector.tensor_scalar_add(o[0:2, 0:2], e[0:2, 0:2], 1.0)
                    else:
                        nc.vector.tensor_copy(o[0:2, 2:4], e[0:2, 0:2])
                nc.sync.dma_start(out_d[0:2, 0:2], o[0:2, 0:2])
    nc.compile()
    return nc


def _build_engine_iso(reps: int, which: str):
    """Isolated engine-stream benchmarks at real sizes."""
    nc = bacc.Bacc(trn_type="TRN2")
    blob_d = nc.dram_tensor("blob", [128, BLOB_W], F32R, kind="ExternalInput")
    xb_d = nc.dram_tensor("xb", [128, N], F32R, kind="ExternalInput")
    out_d = nc.dram_tensor("out", [NH, 128], F32, kind="ExternalOutput")
    BF16 = mybir.dt.bfloat16
    with tile.TileContext(nc) as tc:
        with tc.tile_pool(name="sb", bufs=1) as sb, tc.tile_pool(
            name="wk", bufs=1
        ) as wk, tc.tile_pool(name="ps", bufs=2, space="PSUM") as ps:
            xb = sb.tile([128, N], F32R, name="xb")
            for q in range(4):
                nc.sync.dma_start(xb[:, 1024 * q : 1024 * (q + 1)], xb_d[:, 1024 * q : 1024 * (q + 1)])
            xbb = sb.tile([128, N], BF16, name="xbb")
            nc.vector.tensor_copy(xbb[:, 0:2048], xb[:, 0:2048].bitcast(F32))
            nc.vector.tensor_copy(xbb[:, 2048:4096], xb[:, 2048:4096].bitcast(F32))
            o = sb.tile([128, 64], F32, name="o")
            stp = ps.tile([128, HALF], F32, name="stp", tag="stp", bufs=1)
            nc.tensor.matmul(stp[:, 0:512], xb[0:64, 0:128], xb[0:64, 0:512], start=True, stop=True)
            nc.tensor.matmul(stp[:, 512:1024], xb[0:64, 0:128], xb[0:64, 512:1024], start=True, stop=True)
            with tc.For_i(0, reps, 1, hint_engines=(mybir.EngineType.PE, mybir.EngineType.Activation)):
                if which == "exponly":
                    for j in range(64):
                        ex = wk.tile([128, HALF], F32R, name=f"ex{j}", tag="ex", bufs=4)
                        nc.scalar.activation(ex[:], stp[:], Exp, scale=0.0)
                        if j % 16 == 15:
                            nc.vector.tensor_copy(o[0:1, 0:1], ex[0:1, 0:1].bitcast(F32))
                elif which in ("mmonly", "mmbf16"):
                    dt_att = F32R if which == "mmonly" else BF16
                    src_t = xb if which == "mmonly" else xbb
                    acc = ps.tile([65, HALF], F32, name="acc", tag="acc", bufs=1)
                    for j in range(64):
                        st = ps.tile([128, HALF], F32, name=f"st{j}", tag="st", bufs=2)
                        lt = src_t[0:64, 128 * (j % 32) : 128 * (j % 32) + 128]
                        for k in range(2):
                            nc.tensor.matmul(
                                st[:, 512 * k : 512 * (k + 1)],
                                lt, src_t[0:64, 512 * k : 512 * (k + 1)],
                                start=True, stop=True,
                            )
                        for k in range(2):
                            nc.tensor.matmul(
                                acc[:, 512 * k : 512 * (k + 1)],
                                src_t[:, 65 * (j % 32) : 65 * (j % 32) + 65],
                                src_t[:, 512 * k : 512 * (k + 1)],
                                start=(j == 0), stop=(j == 63),
                            )
                        if j % 8 == 7:
                            nc.vector.tensor_copy(o[0:1, 1:2], st[0:1, 0:1])
                    nc.vector.tensor_copy(o[0:2, 2:4], acc[0:2, 0:2])
                nc.sync.dma_start(out_d[0:2, 0:2], o[0:2, 0:2])
    nc.compile()
    return nc


def _build(reps: int = 1, variant: str = "full"):
    BF16 = mybir.dt.bfloat16
    FP16 = mybir.dt.float16
    # "full" = mixed precision: fp16 theta/phi (10-bit mantissa, small values;
    # S errors get exponentiated so precision matters), bf16 exp/g (needs the
    # f32 exponent range, linearly averaged so rounding mostly cancels).
    if variant == "bf16":
        STT = GEX = BF16
    elif variant == "f32r":
        STT = GEX = F32R
    else:
        STT, GEX = FP16, BF16
    ATT = STT
    nc = bacc.Bacc(trn_type="TRN2")
    blob_d = nc.dram_tensor("blob", [128, BLOB_W], F32R, kind="ExternalInput")
    xb_d = nc.dram_tensor("xb", [128, N], F32R, kind="ExternalInput")
    out_d = nc.dram_tensor("out", [NH, 128], F32, kind="ExternalOutput")

    with tile.TileContext(nc) as tc:
        with tc.tile_pool(name="sb", bufs=1) as sb, tc.tile_pool(
            name="wk", bufs=1
        ) as wk, tc.tile_pool(name="ps", bufs=2, space="PSUM") as ps, tc.tile_pool(
            name="psa", bufs=1, space="PSUM"
        ) as psa:
            blob = sb.tile([128, BLOB_W], F32R, name="blob")
            xb = sb.tile([128, N], F32R, name="xb")
            gta = sb.tile([128, 65 * NCHUNK], GEX, name="gta")
            th2 = sb.tile([128, NH], STT, name="th2")
            ph2 = sb.tile([128, N], STT, name="ph2")
            if variant == "nodma":
                # mark input tiles written so Tile's release check passes
                nc.vector.memset(blob[:, 0:2].bitcast(F32), 1.0)
                nc.vector.memset(xb[:, 0:2].bitcast(F32), 1.0)
                nc.vector.memset(gta[:, 0:2].bitcast(F32), 1.0)
            # repeated body (reps>1 only for slope timing); reps<0 uses a
            # hardware For_i loop of -reps iterations (one body emission).
            import contextlib

            def rep_ctxs():
                if reps >= 1:
                    return [contextlib.nullcontext(i) for i in range(reps)]
                return [
                    tc.For_i(
                        0,
                        -reps,
                        1,
                        hint_engines=(
                            mybir.EngineType.PE,
                            mybir.EngineType.Activation,
                            mybir.EngineType.DVE,
                            mybir.EngineType.SP,
                        ),
                    )
                ]

            for _ctx in rep_ctxs():
              with _ctx:
                # DMA order tuned so the attention pipeline can start ASAP:
                # consts -> xa half 0 -> first xb cols (phi_0) -> the rest.
                  if variant != "nodma":
                      nc.sync.dma_start(blob[:, 0:CONST_W], blob_d[:, 0:CONST_W])
                      nc.sync.dma_start(blob[:, XA : XA + 1024], blob_d[:, XA : XA + 1024])
                      nc.sync.dma_start(xb[:, 0:512], xb_d[:, 0:512])
                      nc.sync.dma_start(xb[:, 512:1536], xb_d[:, 512:1536])
                      nc.sync.dma_start(
                          blob[:, XA + 1024 : BLOB_W], blob_d[:, XA + 1024 : BLOB_W]
                      )
                      ones_cols = gta[:].rearrange("p (j c) -> p j c", c=65)[:, :, 64:65]
                      if variant == "f32r":
                          nc.sync.dma_start(ones_cols, blob_d[:, ONES32 : ONES32 + 32])
                      else:
                          nc.vector.memset(ones_cols, 1.0)
                      nc.sync.dma_start(xb[:, 1536:2560], xb_d[:, 1536:2560])
                      nc.sync.dma_start(xb[:, 2560:3584], xb_d[:, 2560:3584])
                      nc.sync.dma_start(xb[:, 3584:4096], xb_d[:, 3584:4096])
                  if variant == "dmaonly":
                      continue

                  bth = blob[:, BTH : BTH + 1].bitcast(F32)
                  bph = blob[:, BPH : BPH + 1].bitcast(F32)

                  # --- observer preamble ------------------------------------------
                  # Reduce split-wait overhead: PE/DVE observe input-DMA semaphores
                  # once via dummy ops writing corners that real ops overwrite.
                  pth0 = ps.tile([128, 512], F32, name="pth0", tag="aux", bufs=2)
                  nc.tensor.matmul(
                      pth0[0:1, 0:2], blob[0:1, 0:1], blob[0:1, 0:2], start=True, stop=True
                  )
                  nc.tensor.matmul(
                      pth0[0:1, 2:4], xb[0:1, 0:1], xb[0:1, 0:2], start=True, stop=True
                  )
                  dscr = wk.tile([1, 2], F32R, name="dscr", tag="dscr")
                  nc.vector.tensor_copy(dscr[:], blob[0:1, 0:2])

                  # --- theta / phi projections ------------------------------------
                  # th2/ph2 hold the projection duplicated on partitions 0:64 and
                  # 64:128 so both PE row-groups can read weights/rhs locally.
                  def emit_theta(k):
                      if variant == "noproj":
                          return
                      pth = (
                          pth0
                          if k == 0
                          else ps.tile([128, 512], F32, name=f"pth{k}", tag="aux", bufs=2)
                      )
                      nc.tensor.matmul(
                          pth[:],
                          blob[:, WTH : WTH + 128],
                          blob[:, XA + 512 * k : XA + 512 * (k + 1)],
                          start=True,
                          stop=True,
                      )
                      nc.vector.tensor_scalar_add(
                          th2[:, 512 * k : 512 * (k + 1)], pth[:], bth
                      )

                  # k=0,1 cover half 0; k=2,3 (which wait on the later xa DMA) are
                  # deferred into the loop so they don't block the PE FIFO.
                  emit_theta(0)
                  emit_theta(1)

                  def emit_phi(k):
                      if variant == "noproj":
                          return
                      pph = ps.tile([128, 512], F32, name=f"pph{k}", tag="aux", bufs=2)
                      nc.tensor.matmul(
                          pph[:],
                          blob[:, WPH : WPH + 128],
                          xb[:, 512 * k : 512 * (k + 1)],
                          start=True,
                          stop=True,
                      )
                      nc.vector.tensor_scalar_add(
                          ph2[:, 512 * k : 512 * (k + 1)], pph[:], bph
                      )

                  emit_phi(0)
                  emit_phi(1)

                  # gta: g^T in [m, ci] layout, chunk-major with a ones column:
                  # columns [65j, 65j+64) = g^T rows for m-chunk j, column 65j+64 = 1.
                  # Its matmuls are spread one-per-chunk through the half-0 loop so
                  # the injected PE work never stalls the exp pipeline.
                  pgs = {}
                  use16 = variant not in ("f32r",)
                  if use16:
                      xbb = sb.tile([128, N], GEX, name="xbb")
                      wgb = wk.tile([128, 64], GEX, name="wgb", tag="wgb", bufs=1)
                      nc.vector.tensor_copy(wgb[:], blob[:, WG : WG + 64].bitcast(F32))
                      nc.vector.tensor_copy(
                          xbb[:, 0:2048], xb[:, 0:2048].bitcast(F32)
                      )
                      nc.vector.tensor_copy(
                          xbb[:, 2048:4096], xb[:, 2048:4096].bitcast(F32)
                      )

                  def emit_gta_mm(m):
                      if variant == "noproj":
                          return
                      grp = m // 8
                      if grp not in pgs:
                          pgs[grp] = ps.tile([128, 512], F32, name=f"pg{grp}", tag="aux", bufs=2)
                      jj = m % 8
                      if use16:
                          nc.tensor.matmul(
                              pgs[grp][:, 64 * jj : 64 * (jj + 1)],
                              xbb[:, 128 * m : 128 * (m + 1)],
                              wgb[:],
                              start=True,
                              stop=True,
                          )
                      else:
                          nc.tensor.matmul(
                              pgs[grp][:, 64 * jj : 64 * (jj + 1)],
                              xb[:, 128 * m : 128 * (m + 1)],
                              blob[:, WG : WG + 64],
                              start=True,
                              stop=True,
                          )

                  def emit_gta_copy(grp):
                      if variant == "noproj":
                          return
                      pg = pgs[grp]
                      src = pg[:].rearrange("p (j c) -> p j c", c=64)
                      dst = gta[:, 65 * 8 * grp : 65 * 8 * (grp + 1)].rearrange(
                          "p (j c) -> p j c", c=65
                      )[:, :, 0:64]
                      nc.vector.tensor_copy(dst, src)

                  for m in range(8):
                      emit_gta_mm(m)
                  emit_gta_copy(0)
                  emit_gta_mm(8)
                  emit_gta_mm(9)

                  # --- attention main loop ----------------------------------------
                  ys = {}

                  def emit_tail_rest(h):
                      # y[0:64] = unnormalized z^T pre-wout, y[64] = softmax denom.
                      y = ys[h]
                      denp = ps.tile([128, 8], F32, name=f"denp{h}", tag="aux", bufs=2)
                      for t in range(8):
                          nc.tensor.transpose(
                              denp[:, t : t + 1],
                              y[64:65, 128 * t : 128 * (t + 1)].bitcast(F32),
                              blob[64:65, ONE : ONE + 1].bitcast(F32),
                          )
                      r = wk.tile([128, 8], F32, name=f"r{h}", tag="r", bufs=2)
                      nc.vector.reciprocal(r[:], denp[:])

                      ztn = wk.tile([128, HALF], F32, name=f"ztn{h}", tag="ztn", bufs=2)
                      for t in range(8):
                          zt = ps.tile([128, 128], F32, name=f"zt{h}_{t}", tag="aux", bufs=2)
                          nc.tensor.matmul(
                              zt[:],
                              y[0:64, 128 * t : 128 * (t + 1)],
                              blob[0:64, WO : WO + 128],
                              start=True,
                              stop=True,
                          )
                          nc.vector.tensor_scalar_mul(
                              ztn[:, 128 * t : 128 * (t + 1)], zt[:], r[:, t : t + 1]
                          )
                      # z^T tiles straight to DRAM in one DMA; host transposes on
                      # unshard. dst rows 128t+p <- ztn partition p, col-block t.
                      dst = out_d[HALF * h : HALF * (h + 1), :].rearrange(
                          "(t p) c -> p t c", p=128
                      )
                      src = ztn[:].rearrange("p (t c) -> p t c", c=128)
                      nc.sync.dma_start(dst, src)

                  for h in range(2):
                      acc = psa.tile([65, HALF], F32, name=f"acc{h}", tag="acc")
                      for j in range(NCHUNK):
                          if h == 0 and j in (3, 4):
                              emit_theta(j - 1)  # half-1 theta
                          if h == 0 and j >= 2 and (j + 6) % 4 == 0 and (j + 6) // 4 < 8:
                              emit_phi((j + 6) // 4)  # phi_k ready by S^T chunk 4k
                          if h == 0 and 1 <= j <= 22:
                              m = j + 9
                              emit_gta_mm(m)
                              if m % 8 == 7:
                                  emit_gta_copy(m // 8)
                          if h == 1 and j == 4:
                              emit_tail_rest(0)
                          rg = 64 * (j % 2)  # PE row-group: even/odd chunks overlap
                          st = ps.tile([128, HALF], F32, name=f"st{h}_{j}", tag="st")
                          lt = ph2[rg : rg + 64, 128 * j : 128 * (j + 1)]
                          for k in range(2):
                              nc.tensor.matmul(
                                  st[:, 512 * k : 512 * (k + 1)],
                                  lt,
                                  th2[
                                      rg : rg + 64,
                                      HALF * h + 512 * k : HALF * h + 512 * (k + 1),
                                  ],
                                  start=True,
                                  stop=True,
                                  tile_position=(rg, 0),
                              )
                          ex = wk.tile([128, HALF], GEX, name=f"ex{h}_{j}", tag="ex", bufs=4)
                          nc.scalar.activation(ex[:], st[:], Exp)
                          for k in range(2):
                              nc.tensor.matmul(
                                  acc[:, 512 * k : 512 * (k + 1)],
                                  gta[:, 65 * j : 65 * j + 65],
                                  ex[:, 512 * k : 512 * (k + 1)],
                                  start=(j == 0),
                                  stop=(j == NCHUNK - 1),
                              )

                      y = wk.tile([65, HALF], F32R, name=f"y{h}", tag="y", bufs=2)
                      nc.vector.tensor_copy(y[:], acc[:])
                      ys[h] = y
                  emit_tail_rest(1)

    nc.compile()
    return nc


def _fold(w, b, g, beta, m, v):
    """Fold inference BatchNorm into 1x1-conv weight/bias."""
    w = np.asarray(w, np.float64)
    scale = np.asarray(g, np.float64) / np.sqrt(np.asarray(v, np.float64) + EPS)
    wf = w * scale[:, None]
    bf = (np.asarray(b, np.float64) - np.asarray(m, np.float64)) * scale + np.asarray(
        beta, np.float64
    )
    return wf, bf


def prep_in_maps(inputs):
    """Host-side prep: fold BN, build per-core input blobs (shared with bench)."""
    x1 = np.ascontiguousarray(np.asarray(inputs["x1"], np.float32))
    x2 = np.ascontiguousarray(np.asarray(inputs["x2"], np.float32))

    wth, bth = _fold(
        inputs["theta_w"], inputs["theta_b"], inputs["theta_g"],
        inputs["theta_beta"], inputs["theta_m"], inputs["theta_v"],
    )
    wph, bph = _fold(
        inputs["phi_w"], inputs["phi_b"], inputs["phi_g"],
        inputs["phi_beta"], inputs["phi_m"], inputs["phi_v"],
    )
    wg, bg = _fold(
        inputs["g_w"], inputs["g_b"], inputs["g_g"],
        inputs["g_beta"], inputs["g_m"], inputs["g_v"],
    )
    wo, bo = _fold(
        inputs["wout_w"], inputs["wout_b"], inputs["wout_g"],
        inputs["wout_beta"], inputs["wout_m"], inputs["wout_v"],
    )
    cb = (wo @ bg + bo).astype(np.float32)  # absorbed g bias + wout bias

    const = np.zeros((128, CONST_W), np.float32)
    const[:, WTH : WTH + 64] = wth.T.astype(np.float32)
    const[:, WTH + 64 : WTH + 128] = wth.T.astype(np.float32)
    const[:, WPH : WPH + 64] = wph.T.astype(np.float32)
    const[:, WPH + 64 : WPH + 128] = wph.T.astype(np.float32)
    const[:, WG : WG + 64] = wg.T.astype(np.float32)
    const[0:64, WO : WO + 128] = wo.T.astype(np.float32)
    const[0:64, BTH] = bth.astype(np.float32)
    const[64:128, BTH] = bth.astype(np.float32)
    const[0:64, BPH] = bph.astype(np.float32)
    const[64:128, BPH] = bph.astype(np.float32)
    const[:, ONE] = 1.0
    const[:, ONES32 : ONES32 + 32] = 1.0

    in_maps = []
    for core in range(NCORES):
        b, h = divmod(core, 2)
        xa = x1[b].reshape(C, N)[:, NH * h : NH * (h + 1)]
        blob = np.concatenate([const, xa], axis=1)
        in_maps.append(
            {
                "blob": np.ascontiguousarray(blob),
                "xb": np.ascontiguousarray(x2[b].reshape(C, N)),
            }
        )
    return in_maps, cb, x1


def kernel(**inputs) -> np.ndarray:
    in_maps, cb, x1 = prep_in_maps(inputs)

    if _os.environ.get("KLDW", "0") == "1":
        _enable_ldw_opt()
    import os
    kvar = os.environ.get("KVAR", "full")
    if _CACHE.get("kvar") != kvar:
        _CACHE["nc"] = _build(variant=kvar)
        _CACHE["kvar"] = kvar
    nc = _CACHE["nc"]

    kw = dict(_CACHE.get("run_kwargs", {}))
    res = run_bass_kernel_spmd(nc, in_maps, core_ids=list(range(NCORES)), **kw)
    _CACHE["last_results"] = res

    out = np.empty((B, 2 * C, H, W), np.float32)
    for core in range(NCORES):
        b, h = divmod(core, 2)
        zt = res.results[core]["out"]  # [2048, 128] = z^T (unbias'd)
        out[b, 0:C].reshape(C, N)[:, NH * h : NH * (h + 1)] = zt.T + cb[:, None]
    out[:, C:] = x1
    return out



# revision 14
# speedup vs baseline: 1.1161x; 1.1161x over previous
"""Trainium2 Bass kernel for nn_AttentionSlice (non-local attention block).

Reference computation (B=4, C=128, Ci=64, H=W=64, N=H*W=4096):
  theta = BN(conv1x1(x1))   [B, Ci, N]
  phi   = BN(conv1x1(x2))   [B, Ci, N]
  g     = BN(conv1x1(x2))   [B, Ci, N]
  attn  = softmax(theta^T @ phi, axis=-1)          [B, N, N]
  out   = BN(conv1x1(attn @ g^T))                  [B, C, H, W]
  return concat([out, x1], axis=1)                 [B, 2C, H, W]

Sharding: 8 cores = 4 batch samples x 2 halves of the N attention rows.
Each core computes a [2048, 4096] attention block; no cross-core comms.

Device-side design (per core, n = this core's 2048 attention rows):
  - BatchNorm folded into conv weights on the host.
  - S^T is computed chunk-wise in [m, n] layout (m = key position on
    partitions) so softmax needs NO transposes: exp is applied directly
    (S max ~66 < 88, so no max-subtraction is needed for fp32 exp), and
    the softmax denominator falls out of the attn@g^T matmul via an
    appended ones-column on g^T. Division is deferred through the final
    1x1 conv (scaling commutes with the channel contraction).
  - Mixed precision on the hot path (measured fastest on HW): fp16 for
    theta/phi (10-bit mantissa — S errors are exponentiated by softmax so
    precision matters; values are small so fp16 range suffices), bf16 for
    exp(S) and g^T (need fp32-scale exponent range; they enter a weighted
    average so rounding mostly cancels), float32r projections, fp32 tail.
    End-to-end L2 relative error vs the fp32 reference: ~2e-3.
  - Even/odd m-chunks use PE row-groups 0/1 (tile_position) so their
    S^T matmuls run concurrently on the half-idle (K=64) PE array.
  - Output is returned in z^T layout [n, c]; the host transposes and
    adds the (BN-folded) output bias while unsharding.
"""

import sys

if "/opt/trn_rl_repo" not in sys.path:
    sys.path.insert(0, "/opt/trn_rl_repo")

import numpy as np

import concourse.bacc as bacc
import concourse.mybir as mybir
import concourse.tile as tile
from concourse.bass_utils import run_bass_kernel_spmd
import os as _os


def _enable_ldw_opt():
    """Re-enable walrus LDWEIGHTS elision (skips redundant weight loads when
    consecutive matmuls share lhsT). bass_utils hardcodes it off; it breaks
    For_i-loop codegen so it is applied only for the straight-line kernel."""
    import concourse.bass_utils as _bu

    if getattr(_bu, "_ldw_opt_patched", False):
        return
    _orig_run_command = _bu.run_command

    def _run_command_ldwopt(argv, **kw):
        argv = [
            "--enable-ldw-opt=true" if a == "--enable-ldw-opt=false" else a
            for a in argv
        ]
        return _orig_run_command(argv, **kw)

    _bu.run_command = _run_command_ldwopt
    _bu._ldw_opt_patched = True


if _os.environ.get("KLDW", "0") == "1":
    _enable_ldw_opt()

EPS = 1e-5
B, C, CI, H, W = 4, 128, 64, 64, 64
N = H * W  # 4096
NCORES = 8
NH = N // 2  # 2048 rows of attention per core
HALF = 1024  # n processed per pass (PSUM budget)
NCHUNK = 32  # m chunks of 128

F32 = mybir.dt.float32
F32R = mybir.dt.float32r
I16 = mybir.dt.int16
Exp = mybir.ActivationFunctionType.Exp

# Schraudolph fast-exp on DVE: exp(s) ~= bitcast_bf16(int16(SCH_A*s + SCH_B)).
# One fused mul+add tensor_scalar with fp32->int16 round-to-nearest output;
# bf16 bit layout makes the int a piecewise-linear-mantissa exponential.
# RMS rel err ~1.8%, max ~4%; |s| must stay < ~88 (int16 in (0, 32767)).
SCH_A = 128.0 * 1.4426950408889634  # 128 / ln 2
SCH_B = 16249.0  # 127*128 minus mantissa-linearization correction

# Of each 8 consecutive m-chunks, this many run their exp on DVE (fast
# approx) instead of ACT (exact): balances the two elementwise engines.
DVE_ORDER = (2, 5, 7, 0, 4, 6, 1, 3)

# blob column layout: constants first (small fast DMA), then xa.
WTH = 0  # [128, 128] doubled theta weights (lhsT)
WPH = WTH + 128  # [128, 128] doubled phi weights
WG = WPH + 128  # [128, 64]  g weights (rhs form)
WO = WG + 64  # [128, 128] wout weights, rows 0:64 valid (rhs form)
BTH = WO + 128  # [128, 1] theta bias (doubled)
BPH = BTH + 1  # [128, 1] phi bias (doubled)
ONE = BPH + 1  # [128, 1] all ones
ONES32 = ONE + 1  # [128, 32] ones (DMA'd into gta's ones-columns)
CONST_W = ONES32 + 32  # 483
XA = CONST_W  # [128, 2048] x1 slice
BLOB_W = XA + NH

_CACHE: dict = {}


def _build_skeleton(reps: int):
    """Same rough instruction mix as the real kernel, tiny data: measures
    per-instruction issue/sync overhead on HW."""
    nc = bacc.Bacc(trn_type="TRN2")
    blob_d = nc.dram_tensor("blob", [128, BLOB_W], F32R, kind="ExternalInput")
    xb_d = nc.dram_tensor("xb", [128, N], F32R, kind="ExternalInput")
    out_d = nc.dram_tensor("out", [NH, 128], F32, kind="ExternalOutput")
    with tile.TileContext(nc) as tc:
        with tc.tile_pool(name="sb", bufs=1) as sb, tc.tile_pool(
            name="ps", bufs=4, space="PSUM"
        ) as ps:
            a = sb.tile([128, 64], F32R, name="a")
            nc.sync.dma_start(a[:, 0:32], blob_d[:, 0:32])
            nc.sync.dma_start(a[:, 32:64], blob_d[:, 32:64])
            o = sb.tile([128, 64], F32, name="o")
            with tc.For_i(0, reps, 1, hint_engines=(mybir.EngineType.PE, mybir.EngineType.Activation, mybir.EngineType.DVE)):
                for i in range(8):
                    nc.sync.dma_start(a[:, 8 * i : 8 * i + 8], blob_d[:, 8 * i : 8 * i + 8])
                for jj in range(68):
                    p = ps.tile([128, 16], F32, name=f"p{jj}", tag="p")
                    for k in range(5):
                        nc.tensor.matmul(
                            p[0 : 2 + k, 2 * k : 2 * k + 2],
                            a[0:2, k : k + 2 + k].bitcast(F32R),
                            a[0:2, 8 : 8 + 2].bitcast(F32R),
                            start=True,
                            stop=True,
                        )
                    e = sb.tile([128, 16], F32, name=f"e{jj}", tag="e", bufs=4)
                    nc.scalar.activation(e[0:2, 0:2], p[0:2, 0:2], Exp)
                    if jj % 2 == 0:
                        nc.vector.tensor_scalar_add(o[0:2, 0:2], e[0:2, 0:2], 1.0)
                    else:
                        nc.vector.tensor_copy(o[0:2, 2:4], e[0:2, 0:2])
                nc.sync.dma_start(out_d[0:2, 0:2], o[0:2, 0:2])
    nc.compile()
    return nc


def _build_engine_iso(reps: int, which: str):
    """Isolated engine-stream benchmarks at real sizes."""
    nc = bacc.Bacc(trn_type="TRN2")
    blob_d = nc.dram_tensor("blob", [128, BLOB_W], F32R, kind="ExternalInput")
    xb_d = nc.dram_tensor("xb", [128, N], F32R, kind="ExternalInput")
    out_d = nc.dram_tensor("out", [NH, 128], F32, kind="ExternalOutput")
    BF16 = mybir.dt.bfloat16
    with tile.TileContext(nc) as tc:
        with tc.tile_pool(name="sb", bufs=1) as sb, tc.tile_pool(
            name="wk", bufs=1
        ) as wk, tc.tile_pool(name="ps", bufs=2, space="PSUM") as ps:
            xb = sb.tile([128, N], F32R, name="xb")
            for q in range(4):
                nc.sync.dma_start(xb[:, 1024 * q : 1024 * (q + 1)], xb_d[:, 1024 * q : 1024 * (q + 1)])
            xbb = sb.tile([128, N], BF16, name="xbb")
            nc.vector.tensor_copy(xbb[:, 0:2048], xb[:, 0:2048].bitcast(F32))
            nc.vector.tensor_copy(xbb[:, 2048:4096], xb[:, 2048:4096].bitcast(F32))
            o = sb.tile([128, 64], F32, name="o")
            stp = ps.tile([128, HALF], F32, name="stp", tag="stp", bufs=1)
            nc.tensor.matmul(stp[:, 0:512], xb[0:64, 0:128], xb[0:64, 0:512], start=True, stop=True)
            nc.tensor.matmul(stp[:, 512:1024], xb[0:64, 0:128], xb[0:64, 512:1024], start=True, stop=True)
            with tc.For_i(0, reps, 1, hint_engines=(mybir.EngineType.PE, mybir.EngineType.Activation)):
                if which == "exponly":
                    for j in range(64):
                        ex = wk.tile([128, HALF], F32R, name=f"ex{j}", tag="ex", bufs=4)
                        nc.scalar.activation(ex[:], stp[:], Exp, scale=0.0)
                        if j % 16 == 15:
                            nc.vector.tensor_copy(o[0:1, 0:1], ex[0:1, 0:1].bitcast(F32))
                elif which in ("mmonly", "mmbf16"):
                    dt_att = F32R if which == "mmonly" else BF16
                    src_t = xb if which == "mmonly" else xbb
                    acc = ps.tile([65, HALF], F32, name="acc", tag="acc", bufs=1)
                    for j in range(64):
                        st = ps.tile([128, HALF], F32, name=f"st{j}", tag="st", bufs=2)
                        lt = src_t[0:64, 128 * (j % 32) : 128 * (j % 32) + 128]
                        for k in range(2):
                            nc.tensor.matmul(
                                st[:, 512 * k : 512 * (k + 1)],
                                lt, src_t[0:64, 512 * k : 512 * (k + 1)],
                                start=True, stop=True,
                            )
                        for k in range(2):
                            nc.tensor.matmul(
                                acc[:, 512 * k : 512 * (k + 1)],
                                src_t[:, 65 * (j % 32) : 65 * (j % 32) + 65],
                                src_t[:, 512 * k : 512 * (k + 1)],
                                start=(j == 0), stop=(j == 63),
                            )
                        if j % 8 == 7:
                            nc.vector.tensor_copy(o[0:1, 1:2], st[0:1, 0:1])
                    nc.vector.tensor_copy(o[0:2, 2:4], acc[0:2, 0:2])
                nc.sync.dma_start(out_d[0:2, 0:2], o[0:2, 0:2])
    nc.compile()
    return nc


def _build(reps: int = 1, variant: str = "full"):
    BF16 = mybir.dt.bfloat16
    FP16 = mybir.dt.float16
    # Mixed precision: fp16 theta/phi (10-bit mantissa, small values; S
    # errors get exponentiated so precision matters), bf16 exp/g (need the
    # f32 exponent range; they enter a weighted average so rounding mostly
    # cancels).
    STT, GEX = FP16, BF16
    # Engine-split knobs (env-tunable for HW sweeps):
    #   KDVE:  of each 8 m-chunks, how many exp on DVE (Schraudolph)
    #   KPACT: how many of the 12 proj bias-adds run on ACT instead of DVE
    #   KYACT: acc->y copies on ACT (1) or DVE (0)
    #   KZACT: z^T normalize-copies on ACT (1) or DVE (0)
    kdve = int(_os.environ.get("KDVE", "3"))
    kpact = int(_os.environ.get("KPACT", "0"))
    kyact = int(_os.environ.get("KYACT", "0"))
    kzact = int(_os.environ.get("KZACT", "0"))
    dve_slots = frozenset(DVE_ORDER[:kdve])
    Copy = mybir.ActivationFunctionType.Copy
    nc = bacc.Bacc(trn_type="TRN2")
    blob_d = nc.dram_tensor("blob", [128, BLOB_W], F32R, kind="ExternalInput")
    xb_d = nc.dram_tensor("xb", [128, N], F32R, kind="ExternalInput")
    out_d = nc.dram_tensor("out", [NH, 128], F32, kind="ExternalOutput")

    with tile.TileContext(nc) as tc:
        with tc.tile_pool(name="sb", bufs=1) as sb, tc.tile_pool(
            name="wk", bufs=1
        ) as wk, tc.tile_pool(name="ps", bufs=2, space="PSUM") as ps, tc.tile_pool(
            name="psa", bufs=1, space="PSUM"
        ) as psa:
            blob = sb.tile([128, BLOB_W], F32R, name="blob")
            xb = sb.tile([128, N], F32R, name="xb")
            gta = sb.tile([128, 65 * NCHUNK], GEX, name="gta")
            th2 = sb.tile([128, NH], STT, name="th2")
            ph2 = sb.tile([128, N], STT, name="ph2")
            if variant == "nodma":
                # mark input tiles written so Tile's release check passes
                nc.vector.memset(blob[:, 0:2].bitcast(F32), 1.0)
                nc.vector.memset(xb[:, 0:2].bitcast(F32), 1.0)
                nc.vector.memset(gta[:, 0:2].bitcast(F32), 1.0)
            # repeated body (reps>1 only for slope timing); reps<0 uses a
            # hardware For_i loop of -reps iterations (one body emission).
            import contextlib

            def rep_ctxs():
                if reps >= 1:
                    return [contextlib.nullcontext(i) for i in range(reps)]
                return [
                    tc.For_i(
                        0,
                        -reps,
                        1,
                        hint_engines=(
                            mybir.EngineType.PE,
                            mybir.EngineType.Activation,
                            mybir.EngineType.DVE,
                            mybir.EngineType.SP,
                        ),
                    )
                ]

            for _ctx in rep_ctxs():
              with _ctx:
                # DMA order tuned so the attention pipeline can start ASAP:
                # consts -> xa half 0 -> first xb cols (phi_0) -> the rest.
                  if variant != "nodma":
                      nc.sync.dma_start(blob[:, 0:CONST_W], blob_d[:, 0:CONST_W])
                      nc.sync.dma_start(blob[:, XA : XA + 1024], blob_d[:, XA : XA + 1024])
                      nc.sync.dma_start(xb[:, 0:512], xb_d[:, 0:512])
                      nc.sync.dma_start(xb[:, 512:1536], xb_d[:, 512:1536])
                      nc.sync.dma_start(
                          blob[:, XA + 1024 : BLOB_W], blob_d[:, XA + 1024 : BLOB_W]
                      )
                      ones_cols = gta[:].rearrange("p (j c) -> p j c", c=65)[:, :, 64:65]
                      nc.vector.memset(ones_cols, 1.0)
                      nc.sync.dma_start(xb[:, 1536:2560], xb_d[:, 1536:2560])
                      nc.sync.dma_start(xb[:, 2560:3584], xb_d[:, 2560:3584])
                      nc.sync.dma_start(xb[:, 3584:4096], xb_d[:, 3584:4096])
                  if variant == "dmaonly":
                      continue

                  bth = blob[:, BTH : BTH + 1].bitcast(F32)
                  bph = blob[:, BPH : BPH + 1].bitcast(F32)

                  # --- observer preamble ------------------------------------------
                  # Reduce split-wait overhead: PE/DVE observe input-DMA semaphores
                  # once via dummy ops writing corners that real ops overwrite.
                  pth0 = ps.tile([128, 512], F32, name="pth0", tag="aux", bufs=2)
                  nc.tensor.matmul(
                      pth0[0:1, 0:2], blob[0:1, 0:1], blob[0:1, 0:2], start=True, stop=True
                  )
                  nc.tensor.matmul(
                      pth0[0:1, 2:4], xb[0:1, 0:1], xb[0:1, 0:2], start=True, stop=True
                  )
                  dscr = wk.tile([1, 2], F32R, name="dscr", tag="dscr")
                  nc.vector.tensor_copy(dscr[:], blob[0:1, 0:2])

                  # --- theta / phi projections ------------------------------------
                  # th2/ph2 hold the projection duplicated on partitions 0:64 and
                  # 64:128 so both PE row-groups can read weights/rhs locally.
                  def emit_theta(k):
                      if variant == "noproj":
                          return
                      pth = (
                          pth0
                          if k == 0
                          else ps.tile([128, 512], F32, name=f"pth{k}", tag="aux", bufs=2)
                      )
                      nc.tensor.matmul(
                          pth[:],
                          blob[:, WTH : WTH + 128],
                          blob[:, XA + 512 * k : XA + 512 * (k + 1)],
                          start=True,
                          stop=True,
                      )
                      if 8 + k < kpact:  # theta adds are slots 8..11
                          nc.scalar.activation(
                              th2[:, 512 * k : 512 * (k + 1)], pth[:], Copy, bias=bth
                          )
                      else:
                          nc.vector.tensor_scalar_add(
                              th2[:, 512 * k : 512 * (k + 1)], pth[:], bth
                          )

                  # k=0,1 cover half 0; k=2,3 (which wait on the later xa DMA) are
                  # deferred into the loop so they don't block the PE FIFO.
                  emit_theta(0)
                  emit_theta(1)

                  def emit_phi(k):
                      if variant == "noproj":
                          return
                      pph = ps.tile([128, 512], F32, name=f"pph{k}", tag="aux", bufs=2)
                      nc.tensor.matmul(
                          pph[:],
                          blob[:, WPH : WPH + 128],
                          xb[:, 512 * k : 512 * (k + 1)],
                          start=True,
                          stop=True,
                      )
                      if k < kpact:  # phi adds are slots 0..7
                          nc.scalar.activation(
                              ph2[:, 512 * k : 512 * (k + 1)], pph[:], Copy, bias=bph
                          )
                      else:
                          nc.vector.tensor_scalar_add(
                              ph2[:, 512 * k : 512 * (k + 1)], pph[:], bph
                          )

                  emit_phi(0)
                  emit_phi(1)

                  # gta: g^T in [m, ci] layout, chunk-major with a ones column:
                  # columns [65j, 65j+64) = g^T rows for m-chunk j, column 65j+64 = 1.
                  # Its matmuls are spread one-per-chunk through the half-0 loop so
                  # the injected PE work never stalls the exp pipeline.
                  # g^T matmuls read xb/weights as f32r directly (1 cyc/row on
                  # PE only at free>=256; at N=64 it is 4 cyc/row but PE has
                  # slack) — frees the DVE bf16-cast ops for exp work.
                  pgs = {}

                  def emit_gta_mm(m):
                      if variant == "noproj":
                          return
                      grp = m // 8
                      if grp not in pgs:
                          pgs[grp] = ps.tile([128, 512], F32, name=f"pg{grp}", tag="aux", bufs=2)
                      jj = m % 8
                      nc.tensor.matmul(
                          pgs[grp][:, 64 * jj : 64 * (jj + 1)],
                          xb[:, 128 * m : 128 * (m + 1)],
                          blob[:, WG : WG + 64],
                          start=True,
                          stop=True,
                      )

                  def emit_gta_copy(grp):
                      if variant == "noproj":
                          return
                      pg = pgs[grp]
                      src = pg[:].rearrange("p (j c) -> p j c", c=64)
                      dst = gta[:, 65 * 8 * grp : 65 * 8 * (grp + 1)].rearrange(
                          "p (j c) -> p j c", c=65
                      )[:, :, 0:64]
                      nc.vector.tensor_copy(dst, src)

                  for m in range(8):
                      emit_gta_mm(m)
                  emit_gta_copy(0)
                  emit_gta_mm(8)
                  emit_gta_mm(9)

                  # --- attention main loop ----------------------------------------
                  ys = {}

                  def emit_tail_rest(h):
                      # y[0:64] = unnormalized z^T pre-wout, y[64] = softmax denom.
                      y = ys[h]
                      denp = ps.tile([128, 8], F32, name=f"denp{h}", tag="aux", bufs=2)
                      for t in range(8):
                          nc.tensor.transpose(
                              denp[:, t : t + 1],
                              y[64:65, 128 * t : 128 * (t + 1)].bitcast(F32),
                              blob[64:65, ONE : ONE + 1].bitcast(F32),
                          )
                      r = wk.tile([128, 8], F32, name=f"r{h}", tag="r", bufs=2)
                      nc.vector.reciprocal(r[:], denp[:])

                      ztn = wk.tile([128, HALF], F32, name=f"ztn{h}", tag="ztn", bufs=2)
                      for t in range(8):
                          zt = ps.tile([128, 128], F32, name=f"zt{h}_{t}", tag="aux", bufs=2)
                          nc.tensor.matmul(
                              zt[:],
                              y[0:64, 128 * t : 128 * (t + 1)],
                              blob[0:64, WO : WO + 128],
                              start=True,
                              stop=True,
                          )
                          if kzact:
                              nc.scalar.activation(
                                  ztn[:, 128 * t : 128 * (t + 1)],
                                  zt[:],
                                  Copy,
                                  scale=r[:, t : t + 1],
                              )
                          else:
                              nc.vector.tensor_scalar_mul(
                                  ztn[:, 128 * t : 128 * (t + 1)], zt[:], r[:, t : t + 1]
                              )
                      # z^T tiles straight to DRAM in one DMA; host transposes on
                      # unshard. dst rows 128t+p <- ztn partition p, col-block t.
                      dst = out_d[HALF * h : HALF * (h + 1), :].rearrange(
                          "(t p) c -> p t c", p=128
                      )
                      src = ztn[:].rearrange("p (t c) -> p t c", c=128)
                      nc.sync.dma_start(dst, src)

                  for h in range(2):
                      acc = psa.tile([65, HALF], F32, name=f"acc{h}", tag="acc")
                      for j in range(NCHUNK):
                          if h == 0 and j in (3, 4):
                              emit_theta(j - 1)  # half-1 theta
                          if h == 0 and j >= 2 and (j + 6) % 4 == 0 and (j + 6) // 4 < 8:
                              emit_phi((j + 6) // 4)  # phi_k ready by S^T chunk 4k
                          if h == 0 and 1 <= j <= 22:
                              m = j + 9
                              emit_gta_mm(m)
                              if m % 8 == 7:
                                  emit_gta_copy(m // 8)
                          if h == 1 and j == 4:
                              emit_tail_rest(0)
                          rg = 64 * (j % 2)  # PE row-group: even/odd chunks overlap
                          st = ps.tile([128, HALF], F32, name=f"st{h}_{j}", tag="st")
                          lt = ph2[rg : rg + 64, 128 * j : 128 * (j + 1)]
                          for k in range(2):
                              nc.tensor.matmul(
                                  st[:, 512 * k : 512 * (k + 1)],
                                  lt,
                                  th2[
                                      rg : rg + 64,
                                      HALF * h + 512 * k : HALF * h + 512 * (k + 1),
                                  ],
                                  start=True,
                                  stop=True,
                                  tile_position=(rg, 0),
                              )
                          # exp(st): split across both elementwise engines —
                          # ACT does exact LUT exp, DVE does Schraudolph into
                          # the bf16 bit pattern via int16 round. Both write
                          # the same int16 tile type; matmul reads it as bf16.
                          ex = wk.tile([128, HALF], I16, name=f"ex{h}_{j}", tag="ex", bufs=4)
                          if (j % 8) in dve_slots:
                              nc.vector.tensor_scalar(
                                  ex[:],
                                  st[:],
                                  SCH_A,
                                  SCH_B,
                                  mybir.AluOpType.mult,
                                  mybir.AluOpType.add,
                              )
                          else:
                              nc.scalar.activation(ex[:].bitcast(GEX), st[:], Exp)
                          for k in range(2):
                              nc.tensor.matmul(
                                  acc[:, 512 * k : 512 * (k + 1)],
                                  gta[:, 65 * j : 65 * j + 65],
                                  ex[:, 512 * k : 512 * (k + 1)].bitcast(GEX),
                                  start=(j == 0),
                                  stop=(j == NCHUNK - 1),
                              )

                      y = wk.tile([65, HALF], F32R, name=f"y{h}", tag="y", bufs=2)
                      if kyact:
                          nc.scalar.activation(y[:].bitcast(F32), acc[:], Copy)
                      else:
                          nc.vector.tensor_copy(y[:], acc[:])
                      ys[h] = y
                  emit_tail_rest(1)

    nc.compile()
    return nc


def _fold(w, b, g, beta, m, v):
    """Fold inference BatchNorm into 1x1-conv weight/bias."""
    w = np.asarray(w, np.float64)
    scale = np.asarray(g, np.float64) / np.sqrt(np.asarray(v, np.float64) + EPS)
    wf = w * scale[:, None]
    bf = (np.asarray(b, np.float64) - np.asarray(m, np.float64)) * scale + np.asarray(
        beta, np.float64
    )
    return wf, bf


def prep_in_maps(inputs):
    """Host-side prep: fold BN, build per-core input blobs (shared with bench)."""
    x1 = np.ascontiguousarray(np.asarray(inputs["x1"], np.float32))
    x2 = np.ascontiguousarray(np.asarray(inputs["x2"], np.float32))

    wth, bth = _fold(
        inputs["theta_w"], inputs["theta_b"], inputs["theta_g"],
        inputs["theta_beta"], inputs["theta_m"], inputs["theta_v"],
    )
    wph, bph = _fold(
        inputs["phi_w"], inputs["phi_b"], inputs["phi_g"],
        inputs["phi_beta"], inputs["phi_m"], inputs["phi_v"],
    )
    wg, bg = _fold(
        inputs["g_w"], inputs["g_b"], inputs["g_g"],
        inputs["g_beta"], inputs["g_m"], inputs["g_v"],
    )
    wo, bo = _fold(
        inputs["wout_w"], inputs["wout_b"], inputs["wout_g"],
        inputs["wout_beta"], inputs["wout_m"], inputs["wout_v"],
    )
    cb = (wo @ bg + bo).astype(np.float32)  # absorbed g bias + wout bias

    const = np.zeros((128, CONST_W), np.float32)
    const[:, WTH : WTH + 64] = wth.T.astype(np.float32)
    const[:, WTH + 64 : WTH + 128] = wth.T.astype(np.float32)
    const[:, WPH : WPH + 64] = wph.T.astype(np.float32)
    const[:, WPH + 64 : WPH + 128] = wph.T.astype(np.float32)
    const[:, WG : WG + 64] = wg.T.astype(np.float32)
    const[0:64, WO : WO + 128] = wo.T.astype(np.float32)
    const[0:64, BTH] = bth.astype(np.float32)
    const[64:128, BTH] = bth.astype(np.float32)
    const[0:64, BPH] = bph.astype(np.float32)
    const[64:128, BPH] = bph.astype(np.float32)
    const[:, ONE] = 1.0
    const[:, ONES32 : ONES32 + 32] = 1.0

    in_maps = []
    for core in range(NCORES):
        b, h = divmod(core, 2)
        xa = x1[b].reshape(C, N)[:, NH * h : NH * (h + 1)]
        blob = np.concatenate([const, xa], axis=1)
        in_maps.append(
            {
                "blob": np.ascontiguousarray(blob),
                "xb": np.ascontiguousarray(x2[b].reshape(C, N)),
            }
        )
    return in_maps, cb, x1


def kernel(**inputs) -> np.ndarray:
    in_maps, cb, x1 = prep_in_maps(inputs)

    if _os.environ.get("KLDW", "0") == "1":
        _enable_ldw_opt()
    import os
    kvar = os.environ.get("KVAR", "full")
    kkey = (kvar,) + tuple(
        os.environ.get(k, d)
        for k, d in (("KDVE", "3"), ("KPACT", "0"), ("KYACT", "0"), ("KZACT", "0"))
    )
    if _CACHE.get("kvar") != kkey:
        _CACHE["nc"] = _build(variant=kvar)
        _CACHE["kvar"] = kkey
    nc = _CACHE["nc"]

    kw = dict(_CACHE.get("run_kwargs", {}))
    res = run_bass_kernel_spmd(nc, in_maps, core_ids=list(range(NCORES)), **kw)
    _CACHE["last_results"] = res

    out = np.empty((B, 2 * C, H, W), np.float32)
    for core in range(NCORES):
        b, h = divmod(core, 2)
        zt = res.results[core]["out"]  # [2048, 128] = z^T (unbias'd)
        out[b, 0:C].reshape(C, N)[:, NH * h : NH * (h + 1)] = zt.T + cb[:, None]
    out[:, C:] = x1
    return out

